# revision 1
# baseline (speedup 1.0000x reference)
"""Contrastive loss (SupCon-style) on 8 Trainium2 NeuronCores.

Reference (N=8192, D=1024, T=0.1):
    sim = emb @ emb.T / T;  e = exp(sim)
    all_sum_i = sum_j e_ij - e_ii
    pos_sum_i = sum_j e_ij * lab_j - e_ii * lab_i
    loss = mean_{i: lab_i=1} [ log(all_sum_i + eps) - log(pos_sum_i) ]
    (0.0 if n_ref < 2)

Only rows with lab==1 (n_ref ~ 4112) contribute loss; other rows appear only
as columns. Host-side we PERMUTE the embeddings so reference rows come first
(rows and columns permuted identically => the diagonal stays the true
diagonal), then shard the first 8*640=5120 permuted rows across 8 cores
(rows >= n_ref are real non-ref rows, computed fully and masked out).

Per-core COLUMN PACKING (host-side, data-only, SPMD-uniform instructions):
  tile0 = [own 640 rows' cols | 1408 other-ref cols]   (mixed; masked pass)
  tile1 = [ragged leftover ref cols | non-ref fill]    (mixed; masked pass)
  tile2 = [2048 ref cols]                              (pure pos: accum reused)
  tile3 = [2048 non-ref cols]                          (pure neg: no pos work)
Row sums are invariant under column permutation, so all_sum = sum of the 4
per-tile exp accums. pos_sum = masked DVE passes on tiles {0,1} + tile2's
accum. The diagonal block lands at local (k,k) inside tile0 and is excluded
exactly by subtracting BIG pre-exp.

Per (ic, tile): 16 fp8 DoubleRow matmuls -> 4-bank PSUM tile; a few dummy
filler matmuls keep the PE continuously busy so it holds its fast p-state;
ScalarE exp(scale*x) with accum_out emits the tile row-sum for free; DVE
masked pos passes only on tiles 0,1. Final log/mask/reduce on device to one
scalar per core; host sums 8 partials / n_ref.
"""

import numpy as np

import concourse.bass as bass
import concourse.tile as tile
import concourse.mybir as mybir
from concourse import bacc
from concourse.bass_utils import run_bass_kernel_spmd

N, D = 8192, 1024
NCORES = 8
ROWS = 640          # per-core row capacity (5 chunks of 128; covers n_ref<=5120)
P = 128             # partitions
JS = 512            # j-slice width (one PSUM bank of fp32)
NS = 4              # PSUM banks (j slices) per tile -> 2048-wide exp instrs
JT = NS * JS        # j-tile width (2048)
NJT = N // JT       # 4 j tiles
NMIX = 2            # tiles 0,1 are pos/neg-mixed -> masked DVE pass
PURE_POS_JT = 2     # tile2 is all-ref (pos) for every core
ND = D // P         # 8 contraction chunks
IC = ROWS // P      # 5 row chunks per core
FILLER = 4          # dummy matmuls per (ic,jt) to keep PE p-state hot
SCALE = 10.0        # 1 / TEMPERATURE
EPS = 1e-8
BIG = 1e9           # sim[diag] -= BIG before exp => exp -> 0

F32 = mybir.dt.float32
BF16 = mybir.dt.bfloat16
DT_MM = mybir.dt.float8e4

_build_cache = {}


EXP_LN_TABLE = False  # reordering act tables broke numerics on HW


class _Bacc(bacc.Bacc):
    """Prefer the combined exp+ln activation table so the per-rep Exp->Ln
    alternation needs no ACT_TABLE_LOAD swaps (1283ns each)."""

    def insert_act_table_loads(self):
        if not EXP_LN_TABLE:
            return super().insert_act_table_loads()
        import bass_rust as _bass_rust
        from concourse.hw_specs import get_activation_tables

        has_activation = any(
            isinstance(i, mybir.InstActivation)
            for b in self.main_func.blocks
            for i in b.instructions
        )
        if not has_activation:
            return
        tables = list(get_activation_tables(self.m.arch).items())
        pref = [t for t in tables if t[0] == "natural_log_exp_and_others"]
        rest = [t for t in tables if t[0] != "natural_log_exp_and_others"]
        _bass_rust.insert_act_table_loads(self, pref + rest)


def build(reps: int = 1, level: int = 3, dt_mm=None, filler=None,
          rowsum_dve: bool = False):
    """level: 0=mm+exp only, 1=+pos passes, 2=+diag-sub, 3=full."""
    if dt_mm is None:
        dt_mm = DT_MM
    if filler is None:
        filler = FILLER
    key = (reps, level, dt_mm, filler, rowsum_dve)
    if key in _build_cache:
        return _build_cache[key]

    nc = _Bacc("TRN2", target_bir_lowering=False, debug=False)
    embT_d = nc.dram_tensor("embT", [D, N], dt_mm, kind="ExternalInput")
    lab_d = nc.dram_tensor("lab", [NMIX * JT], BF16, kind="ExternalInput")
    labt_d = nc.dram_tensor("labt", [P, IC], F32, kind="ExternalInput")
    partial_d = nc.dram_tensor("partial", [P, IC], F32, kind="ExternalOutput")

    # [D, N] viewed as [p, dc, n] with d = dc*128 + p
    embT = embT_d.ap().rearrange("(dc p) n -> p dc n", p=P)
    lab_bcast_src = bass.AP(tensor=lab_d, offset=0, ap=[[0, P], [1, NMIX * JT]])

    with tile.TileContext(nc) as tc:
        with (
            tc.tile_pool(name="consts", bufs=1) as consts,
            tc.tile_pool(name="rhsp", bufs=4) as rhsp,
            tc.tile_pool(name="expp", bufs=4) as expp,
            tc.tile_pool(name="scrp", bufs=2) as scrp,
            tc.tile_pool(name="stats", bufs=2) as stats,
            tc.tile_pool(name="fin", bufs=2) as fin,
            tc.tile_pool(name="psum", bufs=2, space=bass.MemorySpace.PSUM) as psum,
        ):
            # resident stationary operand: this core's 640 embedding columns
            res = consts.tile([P, ND, ROWS], dt_mm)
            nc.sync.dma_start(out=res, in_=embT[:, :, 0:ROWS])
            # labels broadcast across partitions for the 2 mixed tiles only
            labb = consts.tile([P, NMIX * JT], BF16)
            nc.gpsimd.dma_start(out=labb, in_=lab_bcast_src)
            # per-row label mask in [p, ic] layout
            labt = consts.tile([P, IC], F32)
            nc.sync.dma_start(out=labt, in_=labt_d.ap())
            # BIG * identity (subtracted on the diagonal block pre-exp)
            bigI = consts.tile([P, P], F32)
            nc.gpsimd.memset(bigI, 0.0)
            nc.gpsimd.affine_select(
                out=bigI,
                in_=bigI,
                compare_op=mybir.AluOpType.not_equal,
                fill=BIG,
                base=0,
                pattern=[[-1, P]],
                channel_multiplier=1,
            )

            for rep in range(reps):
                alls = stats.tile([P, IC * NJT], F32, tag="alls")
                poss = stats.tile([P, IC * NMIX], F32, tag="poss")

                use_dr = dt_mm in mybir.MATMUL_PERF_MODE_DTYPES
                for jp in range(NJT):
                    rhs = rhsp.tile([P, ND, JT], dt_mm, tag="rhs")
                    eng = nc.sync if jp % 2 == 0 else nc.gpsimd
                    eng.dma_start(
                        out=rhs, in_=embT[:, :, jp * JT : (jp + 1) * JT]
                    )
                    for ic in range(IC):
                        ps = psum.tile([P, NS, JS], F32, tag="ps")
                        # dummy groups into bank 0: real group's start=True
                        # resets them; they only keep the PE p-state hot
                        for f in range(filler):
                            nc.tensor.matmul(
                                ps[:, 0, :],
                                res[:, 0:2, ic * P : (ic + 1) * P],
                                rhs[:, 0:2, 0:JS],
                                start=True,
                                stop=True,
                                perf_mode=mybir.MatmulPerfMode.DoubleRow,
                                skip_group_check=True,
                            )
                        for s in range(NS):
                            rhs_s = rhs[:, :, s * JS : (s + 1) * JS]
                            if use_dr:
                                for dc2 in range(ND // 2):
                                    nc.tensor.matmul(
                                        ps[:, s, :],
                                        res[:, 2 * dc2 : 2 * dc2 + 2, ic * P : (ic + 1) * P],
                                        rhs_s[:, 2 * dc2 : 2 * dc2 + 2, :],
                                        start=(dc2 == 0),
                                        stop=(dc2 == ND // 2 - 1),
                                        perf_mode=mybir.MatmulPerfMode.DoubleRow,
                                    )
                            else:
                                for dc in range(ND):
                                    nc.tensor.matmul(
                                        ps[:, s, :],
                                        res[:, dc, ic * P : (ic + 1) * P],
                                        rhs_s[:, dc, :],
                                        start=(dc == 0),
                                        stop=(dc == ND - 1),
                                    )
                        # diagonal block: own rows sit at local cols
                        # [ic*128, ic*128+128) inside j-tile 0
                        if level >= 2 and jp == 0:
                            s_d = (ic * P) // JS
                            off = (ic * P) % JS
                            nc.vector.tensor_sub(
                                ps[:, s_d, off : off + P],
                                ps[:, s_d, off : off + P],
                                bigI,
                            )
                        ext = expp.tile([P, JT], BF16, tag="ext")
                        idx = ic * NJT + jp
                        if rowsum_dve:
                            nc.scalar.activation(
                                out=ext,
                                in_=ps.rearrange("p s j -> p (s j)"),
                                func=mybir.ActivationFunctionType.Exp,
                                scale=SCALE,
                            )
                            junk2 = scrp.tile([P, JT], BF16, tag="junk2")
                            nc.vector.tensor_scalar(
                                out=junk2,
                                in0=ext,
                                scalar1=1.0,
                                scalar2=None,
                                op0=mybir.AluOpType.mult,
                                accum_out=alls[:, idx : idx + 1],
                            )
                        else:
                            nc.scalar.activation(
                                out=ext,
                                in_=ps.rearrange("p s j -> p (s j)"),
                                func=mybir.ActivationFunctionType.Exp,
                                scale=SCALE,
                                accum_out=alls[:, idx : idx + 1],
                            )
                        if level < 1 or jp >= NMIX:
                            continue
                        junk = scrp.tile([P, JT], BF16, tag="junk")
                        nc.vector.scalar_tensor_tensor(
                            out=junk,
                            in0=ext,
                            scalar=1.0,
                            in1=labb[:, jp * JT : (jp + 1) * JT],
                            op0=mybir.AluOpType.mult,
                            op1=mybir.AluOpType.mult,
                            accum_out=poss[:, ic * NMIX + jp : ic * NMIX + jp + 1],
                        )

                # ---- per-row loss and partial reduction ----
                asum = fin.tile([P, IC], F32, tag="asum")
                nc.vector.reduce_sum(
                    asum,
                    alls.rearrange("p (ic nj) -> p ic nj", nj=NJT),
                    axis=mybir.AxisListType.X,
                )
                if level < 1:
                    nc.sync.dma_start(out=partial_d.ap(), in_=asum)
                    continue
                # pos = masked passes (tiles 0,1) + tile2 accum (pure pos)
                # sums laid out as combo[:, 0, :]=all, combo[:, 1, :]=pos so a
                # single Ln instruction covers both
                combo = fin.tile([P, 2, IC], F32, tag="combo")
                nc.vector.reduce_sum(
                    combo[:, 1, :],
                    poss.rearrange("p (ic nj) -> p ic nj", nj=NMIX),
                    axis=mybir.AxisListType.X,
                )
                alls_v = alls.rearrange("p (ic nj) -> p ic nj", nj=NJT)
                nc.vector.tensor_add(
                    combo[:, 1, :], combo[:, 1, :], alls_v[:, :, PURE_POS_JT]
                )
                nc.vector.tensor_scalar(
                    out=combo[:, 0, :], in0=asum, scalar1=EPS, scalar2=None,
                    op0=mybir.AluOpType.add,
                )
                lnc = fin.tile([P, 2, IC], F32, tag="lnc")
                nc.scalar.activation(
                    out=lnc,
                    in_=combo,
                    func=mybir.ActivationFunctionType.Ln,
                )
                contrib = fin.tile([P, IC], F32, tag="contrib")
                nc.vector.tensor_sub(contrib, lnc[:, 0, :], lnc[:, 1, :])
                nc.vector.tensor_mul(contrib, contrib, labt)
                nc.sync.dma_start(out=partial_d.ap(), in_=contrib)

    nc.compile()
    _build_cache[key] = nc
    return nc


def make_in_maps(embeddings: np.ndarray, labels: np.ndarray, dt_mm=None):
    if dt_mm is None:
        dt_mm = DT_MM
    emb = np.asarray(embeddings, dtype=np.float32)
    lab_f = np.asarray(labels).astype(np.float32)
    # permute: reference rows first (stable), non-ref after. Rows and columns
    # are permuted identically so the diagonal stays the true diagonal.
    perm = np.argsort(-lab_f, kind="stable")
    emb_p = emb[perm]
    lab_p = lab_f[perm]
    M = int(lab_f.sum())
    assert M <= NCORES * ROWS, "n_ref exceeds row capacity"
    embT = np.ascontiguousarray(emb_p.T).astype(mybir.dt.np(dt_mm))  # [D, N]
    allpos = np.flatnonzero(lab_p > 0)  # = [0, M)
    allneg = np.flatnonzero(lab_p == 0)
    in_maps = []
    for c in range(NCORES):
        own = np.arange(c * ROWS, (c + 1) * ROWS)
        own_set_lo, own_set_hi = c * ROWS, (c + 1) * ROWS
        pos_rest = allpos[(allpos < own_set_lo) | (allpos >= own_set_hi)]
        neg_rest = allneg[(allneg < own_set_lo) | (allneg >= own_set_hi)]
        npos = len(pos_rest)
        # tile0 = own + 1408 pos; tile2 = 2048 pure pos;
        # tile1 = ragged pos leftover + neg fill; tile3 = 2048 pure neg
        f0 = JT - ROWS                      # 1408
        t1_pos = npos - f0 - JT             # ragged leftover (16..656)
        assert 0 <= t1_pos <= JT, f"core {c}: t1_pos={t1_pos}"
        cols = np.concatenate([
            own,
            pos_rest[:f0],
            pos_rest[f0 + JT :],
            neg_rest[: JT - t1_pos],
            pos_rest[f0 : f0 + JT],
            neg_rest[JT - t1_pos :],
        ])
        assert len(cols) == N
        labc = lab_p[cols]
        labt = np.ascontiguousarray(labc[0:ROWS].reshape(IC, P).T)
        in_maps.append(
            {
                "embT": np.ascontiguousarray(embT[:, cols]),
                "lab": labc[: NMIX * JT].astype(mybir.dt.np(BF16)),
                "labt": labt.astype(np.float32),
            }
        )
    return in_maps


def kernel(embeddings: np.ndarray, labels: np.ndarray) -> np.ndarray:
    lab_f = np.asarray(labels).astype(np.float32)
    n_ref = float(lab_f.sum())
    if n_ref < 2:
        return np.float32(0.0)

    nc = build(reps=1)
    in_maps = make_in_maps(embeddings, labels)
    res = run_bass_kernel_spmd(nc, in_maps, core_ids=list(range(NCORES)))
    total = np.float32(0.0)
    for c in range(NCORES):
        total += res.results[c]["partial"].sum(dtype=np.float32)
    loss = total / np.float32(max(n_ref, 1.0))
    return np.asarray(loss, dtype=np.float32)



# revision 5
# speedup vs baseline: 1.3098x; 1.3098x over previous
"""Contrastive loss (SupCon-style) on 8 Trainium2 NeuronCores — v2.

Reference (N=8192, D=1024, T=0.1, normalized embeddings):
    sim = emb @ emb.T / T;  e = exp(sim)
    all_sum_i = sum_j e_ij - e_ii
    pos_sum_i = sum_j e_ij * lab_j - e_ii * lab_i
    loss = mean_{i: lab_i=1} [ log(all_sum_i + eps) - log(pos_sum_i) ]

Structure (v2):
  * Host permutes embeddings ref-first (one global permutation, identical for
    every core) and quantizes once to fp8: embT [D, N].  Core c's stationary
    operand is its 512-row slice resT = embT[:, c*512:(c+1)*512].
  * The diagonal e_ii is NOT removed on device.  The device accumulates plain
    per-tile row sums; the host subtracts exp(10 * ||q(e_i)||^2) computed
    from the same fp8 values (fp8 products are exact in fp32; only the
    reduction order differs).
  * pos_sum needs no label mask on device: ref columns are a global prefix,
    so pos = sum of the first JT_FULL_POS whole-tile sums + one narrow
    window pass of width W = n_ref mod 2048 on tile JT_FULL_POS (DVE).
  * Device rows = 8*512 = 4096; leftover ref rows (n_ref - 4096) are
    computed on the host in f32 (tiny GEMM).  Final log/mask/reduce on host.
  * Device returns [P, NOUT] f32 of raw sums per core.

Per (ic, jt): 16 fp8 DoubleRow matmuls -> 4-bank PSUM tile; ScalarE
exp(10*x) IN-PLACE on PSUM with accum_out emitting the tile row-sum.  The
first and last tiles run exp per 512-wide PSUM bank instead (4 partial sums
each) so the Act stream starts ~4us earlier and the tail exp shrinks.
Dummy warmup matmuls at t=0 ramp the PE p-state while the first DMAs land;
fill0 dummies bridge the jt0 chunk-DMA gaps; a few filler dummies per tile
bridge the Act/PE pace gap.  All DMAs ride the sync queue (HWDGE).
"""

import numpy as np

import concourse.bass as bass
import concourse.tile as tile
import concourse.mybir as mybir
from concourse import bacc
from concourse.bass_utils import run_bass_kernel_spmd

N, D = 8192, 1024
NCORES = 8
P = 128             # partitions
RES = 512           # per-core device rows (4 chunks of 128)
IC = RES // P       # 4 row chunks per core
JS = 512            # j-slice width (one PSUM bank of fp32)
NS = 4              # PSUM banks per tile -> 2048-wide exp instrs
JT = NS * JS        # j-tile width (2048)
NJT = N // JT       # 4 j tiles
ND = D // P         # 8 contraction chunks
SCALE = 10.0        # 1 / TEMPERATURE
EPS = 1e-8

F32 = mybir.dt.float32
DT_MM = mybir.dt.float8e4
F8NP = mybir.dt.np(DT_MM)

WARMUP = 60         # dummy matmuls at t=0 (p-state ramp during DMA wait)
FILL0 = 40          # dummies between (jt0, ic0) chunk groups (DMA gaps)
FILLER = 8          # dummies per (ic, jt) to bridge the Act/PE pace gap

# st column layout: [0:16) whole-tile row sums (idx = ic*NJT + jt; unused for
# the two bank-split tiles), [16:20) window sums, [20:24) first-tile bank
# sums, [24:28) last-tile bank sums
NOUT = IC * NJT + IC + 2 * NS

_build_cache = {}


def build(reps: int = 1, w: int = 0, jtf: int = 2, warmup=None, fill0=None,
          filler=None):
    """w: pos window width on tile jtf (= n_ref % JT); jtf = n_ref // JT."""
    if warmup is None:
        warmup = WARMUP
    if fill0 is None:
        fill0 = FILL0
    if filler is None:
        filler = FILLER
    key = (reps, w, jtf, warmup, fill0, filler)
    if key in _build_cache:
        return _build_cache[key]

    nc = bacc.Bacc("TRN2", target_bir_lowering=False, debug=False)
    resT_d = nc.dram_tensor("resT", [D, RES], DT_MM, kind="ExternalInput")
    embT_d = nc.dram_tensor("embT", [D, N], DT_MM, kind="ExternalInput")
    partial_d = nc.dram_tensor("partial", [P, NOUT], F32, kind="ExternalOutput")

    # [D, X] viewed as [p, dc, x] with d = dc*128 + p
    resT = resT_d.ap().rearrange("(dc p) r -> p dc r", p=P)
    embT = embT_d.ap().rearrange("(dc p) n -> p dc n", p=P)

    with tile.TileContext(nc) as tc:
        with (
            tc.tile_pool(name="consts", bufs=1) as consts,
            tc.tile_pool(name="rhsp", bufs=8) as rhsp,
            tc.tile_pool(name="stats", bufs=2) as stats,
            tc.tile_pool(name="junk", bufs=2) as junkp,
            tc.tile_pool(name="psum", bufs=2, space=bass.MemorySpace.PSUM) as psum,
        ):
            # warmup/filler matmul operands (results never read; only the PE
            # busy-time matters).  Small so the memset is fast.
            wtile = consts.tile([P, 2, P], DT_MM)
            nc.vector.memset(wtile, 0.0)
            # warmup psum target: pool buffer 0 (reused by later real tiles)
            wps = psum.tile([P, NS, JS], F32, tag="ps")

            def dummy_mm(ps_target, n):
                for _ in range(n):
                    nc.tensor.matmul(
                        ps_target[:, 0:P],
                        wtile[:, 0:2, 0:P],
                        wtile[:, 0:2, 0:P],
                        start=True,
                        stop=True,
                        perf_mode=mybir.MatmulPerfMode.DoubleRow,
                        skip_group_check=True,
                    )

            # dummy activation at t=0: pulls the ACT_TABLE_LOAD for Exp off
            # the critical path (runs concurrently with the input DMAs)
            tjunk = junkp.tile([P, 1], F32, tag="tj")
            nc.scalar.activation(
                out=tjunk, in_=wtile[:, 0, 0:1],
                func=mybir.ActivationFunctionType.Exp,
            )

            dummy_mm(wps[:, 0, :], warmup)

            # resident stationary operand: this core's 512 embedding columns
            res = consts.tile([P, ND, RES], DT_MM)
            nc.sync.dma_start(out=res, in_=resT)

            for rep in range(reps):
                st = stats.tile([P, NOUT], F32, tag="st")

                for jt in range(NJT):
                    chunks = []
                    for s in range(NS):
                        rhs = rhsp.tile([P, ND, JS], DT_MM, tag="rhs")
                        j0 = jt * JT + s * JS
                        nc.sync.dma_start(out=rhs, in_=embT[:, :, j0 : j0 + JS])
                        chunks.append(rhs)
                    for ic in range(IC):
                        first = jt == 0 and ic == 0
                        last = jt == NJT - 1 and ic == IC - 1
                        split = first or last
                        ps = psum.tile([P, NS, JS], F32, tag="ps")
                        if filler and not first:
                            dummy_mm(ps[:, 0, :], filler)
                        for s in range(NS):
                            for dc2 in range(ND // 2):
                                nc.tensor.matmul(
                                    ps[:, s, :],
                                    res[:, 2 * dc2 : 2 * dc2 + 2, ic * P : (ic + 1) * P],
                                    chunks[s][:, 2 * dc2 : 2 * dc2 + 2, :],
                                    start=(dc2 == 0),
                                    stop=(dc2 == ND // 2 - 1),
                                    perf_mode=mybir.MatmulPerfMode.DoubleRow,
                                )
                            if split:
                                # per-bank exp: starts the Act stream early
                                # (first tile) / shrinks the tail (last tile)
                                widx = IC * NJT + IC + (NS if last else 0) + s
                                nc.scalar.activation(
                                    out=ps[:, s, :],
                                    in_=ps[:, s, :],
                                    func=mybir.ActivationFunctionType.Exp,
                                    scale=SCALE,
                                    accum_out=st[:, widx : widx + 1],
                                )
                            if first and fill0 and s < NS - 1:
                                dummy_mm(wps[:, 0, :], fill0)
                        if not split:
                            # in-place exp on the 4-bank PSUM tile; accum_out
                            # emits the 2048-wide row sum for free
                            ps_flat = ps.rearrange("p s j -> p (s j)")
                            idx = ic * NJT + jt
                            nc.scalar.activation(
                                out=ps_flat,
                                in_=ps_flat,
                                func=mybir.ActivationFunctionType.Exp,
                                scale=SCALE,
                                accum_out=st[:, idx : idx + 1],
                            )
                        if jt == jtf and w > 0:
                            ps_flat2 = ps.rearrange("p s j -> p (s j)")
                            wj = junkp.tile([P, w], F32, tag="wj")
                            widx = IC * NJT + ic
                            nc.vector.tensor_scalar(
                                out=wj,
                                in0=ps_flat2[:, 0:w],
                                scalar1=1.0,
                                scalar2=0.0,
                                op0=mybir.AluOpType.mult,
                                op1=mybir.AluOpType.add,
                                accum_out=st[:, widx : widx + 1],
                            )
                nc.sync.dma_start(out=partial_d.ap(), in_=st)

    nc.compile()
    _build_cache[key] = nc
    return nc


def make_in_maps(embeddings: np.ndarray, labels: np.ndarray):
    emb = np.asarray(embeddings, dtype=np.float32)
    lab_f = np.asarray(labels).astype(np.float32)
    perm = np.argsort(-lab_f, kind="stable")
    emb_p = emb[perm]
    n_ref = int(lab_f.sum())
    assert n_ref <= NCORES * RES + 1024, "n_ref exceeds device+host capacity"
    embT = np.ascontiguousarray(emb_p.T).astype(F8NP)  # [D, N], one array
    in_maps = []
    for c in range(NCORES):
        in_maps.append(
            {
                "resT": np.ascontiguousarray(embT[:, c * RES : (c + 1) * RES]),
                "embT": embT,
            }
        )
    qf32 = embT.astype(np.float32)
    sq = np.einsum("dn,dn->n", qf32, qf32, dtype=np.float32)  # ||q(e_i)||^2
    ctx = {"emb_p": emb_p, "n_ref": n_ref, "sq": sq}
    return in_maps, ctx


def host_finish(partials, ctx):
    """partials: list of [P, NOUT] per core -> scalar loss (f32)."""
    n_ref = ctx["n_ref"]
    emb_p = ctx["emb_p"]
    sq = ctx["sq"]
    jtf = n_ref // JT
    dev_rows = NCORES * RES
    nb = IC * NJT + IC
    total = np.float64(0.0)
    for c in range(NCORES):
        arr = np.asarray(partials[c], np.float32)
        A = arr[:, : IC * NJT].reshape(P, IC, NJT).copy()
        A[:, 0, 0] = arr[:, nb : nb + NS].sum(axis=1)            # first tile
        A[:, IC - 1, NJT - 1] = arr[:, nb + NS : nb + 2 * NS].sum(axis=1)
        Wv = arr[:, IC * NJT : nb]                               # [P, IC]
        rows = c * RES + np.arange(IC)[None, :] * P + np.arange(P)[:, None]
        corr = np.exp(SCALE * sq[rows])                          # [P, IC]
        all_r = A.sum(axis=2) - corr
        pos_r = A[:, :, :jtf].sum(axis=2) + Wv - corr
        mask = rows < n_ref
        if not mask.any():
            continue
        contrib = np.where(
            mask,
            np.log(np.maximum(all_r, 1e-30) + EPS) - np.log(np.maximum(pos_r, 1e-30)),
            0.0,
        )
        total += contrib.sum(dtype=np.float64)
    if n_ref > dev_rows:
        hr = np.arange(dev_rows, n_ref)
        sim_h = (emb_p[hr] @ emb_p.T) * SCALE
        e_h = np.exp(sim_h)
        diag = e_h[np.arange(len(hr)), hr]
        all_h = e_h.sum(axis=1) - diag
        pos_h = e_h[:, :n_ref].sum(axis=1) - diag
        total += (np.log(all_h + EPS) - np.log(pos_h)).sum(dtype=np.float64)
    loss = total / max(n_ref, 1)
    return np.float32(loss)


def kernel(embeddings: np.ndarray, labels: np.ndarray) -> np.ndarray:
    lab_f = np.asarray(labels).astype(np.float32)
    n_ref = float(lab_f.sum())
    if n_ref < 2:
        return np.float32(0.0)

    in_maps, ctx = make_in_maps(embeddings, labels)
    w = ctx["n_ref"] % JT
    jtf = ctx["n_ref"] // JT
    nc = build(reps=1, w=w, jtf=jtf)
    res = run_bass_kernel_spmd(nc, in_maps, core_ids=list(range(NCORES)))
    partials = [res.results[c]["partial"] for c in range(NCORES)]
    return np.asarray(host_finish(partials, ctx), dtype=np.float32)


# revision 23
# speedup vs baseline: 1.3492x; 1.0301x over previous
"""Contrastive loss (SupCon-style) on 8 Trainium2 NeuronCores — v2.

Reference (N=8192, D=1024, T=0.1, normalized embeddings):
    sim = emb @ emb.T / T;  e = exp(sim)
    all_sum_i = sum_j e_ij - e_ii
    pos_sum_i = sum_j e_ij * lab_j - e_ii * lab_i
    loss = mean_{i: lab_i=1} [ log(all_sum_i + eps) - log(pos_sum_i) ]

Structure (v2):
  * Host permutes embeddings ref-first (one global permutation, identical for
    every core) and quantizes once to fp8: embT [D, N].  Core c's stationary
    operand is its 512-row slice resT = embT[:, c*512:(c+1)*512].
  * The diagonal e_ii is NOT removed on device.  The device accumulates plain
    per-tile row sums; the host subtracts exp(10 * ||q(e_i)||^2) computed
    from the same fp8 values (fp8 products are exact in fp32; only the
    reduction order differs).
  * pos_sum needs no label mask on device: ref columns are a global prefix,
    so pos = sum of the first JT_FULL_POS whole-tile sums + one narrow
    window pass of width W = n_ref mod 2048 on tile JT_FULL_POS (DVE).
  * Device rows = 8*512 = 4096; leftover ref rows (n_ref - 4096) are
    computed on the host in f32 (tiny GEMM).  Final log/mask/reduce on host.
  * Device returns [P, NOUT] f32 of raw sums per core.

Per (ic, jt): 16 fp8 DoubleRow matmuls -> 4-bank PSUM tile; ScalarE
exp(10*x) IN-PLACE on PSUM with accum_out emitting the tile row-sum.  The
first and last tiles run exp per 512-wide PSUM bank instead (4 partial sums
each) so the Act stream starts ~4us earlier and the tail exp shrinks.
Dummy warmup matmuls at t=0 ramp the PE p-state while the first DMAs land;
fill0 dummies bridge the jt0 chunk-DMA gaps; a few filler dummies per tile
bridge the Act/PE pace gap.  All DMAs ride the sync queue (HWDGE).
"""

import numpy as np

import concourse.bass as bass
import concourse.tile as tile
import concourse.mybir as mybir
from concourse import bacc
from concourse.bass_utils import run_bass_kernel_spmd

N, D = 8192, 1024
NCORES = 8
P = 128             # partitions
RES = 512           # per-core device rows (4 chunks of 128)
IC = RES // P       # 4 row chunks per core
JS = 512            # j-slice width (one PSUM bank of fp32)
NS = 4              # PSUM banks per tile -> 2048-wide exp instrs
JT = NS * JS        # j-tile width (2048)
NJT = N // JT       # 4 j tiles
ND = D // P         # 8 contraction chunks
SCALE = 10.0        # 1 / TEMPERATURE
EPS = 1e-8

F32 = mybir.dt.float32
DT_MM = mybir.dt.float8e4
F8NP = mybir.dt.np(DT_MM)

# HW-measured: steady-state dummy matmuls are a net loss (each pays the
# full 256-column DoubleRow LDWEIGHTS, ~213ns) — but a warmup train during
# the initial DMA wait warms the PE throttle state for ~-4us.  With dedupe
# the warmup dummies share one weight load.
WARMUP = 56         # dummy matmuls at t=0 (overlap the first DMAs)
FILL0 = 0           # dummies between (jt0, ic0) chunk groups
FILLER = 0          # dummies per (ic, jt)

# st column layout: [0:16) whole-tile row sums (idx = ic*NJT + jt; unused for
# the two bank-split tiles), [16:20) window sums, [20:24) first-tile bank
# sums, [24:28) last-tile bank sums
NOUT = IC * NJT + IC + 2 * NS

_build_cache = {}


def dedupe_ldweights(nc):
    """Remove InstLdweights that reload the exact weights already resident
    in the PE array (same source AP + perf mode as the previous load),
    merging their waits/updates into the next PE instruction."""
    PE = mybir.EngineType.PE
    total = 0
    for b in nc.main_func.blocks:
        keep = []
        last_sig = None
        pend_w, pend_u = [], []
        removed = 0
        for i in b.instructions:
            if isinstance(i, mybir.InstLdweights):
                sig = (str(i.ins[0]), str(i.perf_mode))
                if sig == last_sig:
                    si = i.sync_info
                    if si is not None:
                        pend_w += list(si.on_wait)
                        pend_u += list(si.on_update)
                    removed += 1
                    continue
                last_sig = sig
            if (pend_w or pend_u) and getattr(i, "engine", None) == PE:
                si = i.sync_info
                if si is None:
                    i.sync_info = mybir.SyncInfo(on_wait=pend_w, on_update=pend_u)
                else:
                    si.on_wait = list(si.on_wait) + pend_w
                    si.on_update = list(si.on_update) + pend_u
                pend_w, pend_u = [], []
            keep.append(i)
        if removed:
            assert not pend_w and not pend_u, "dangling sync from removed ldw"
            n_old = len(b.instructions)
            for _ in range(n_old):
                b.instructions.pop()
            for i in keep:
                b.instructions.append(i)
            total += removed
    return total


def build(reps: int = 1, w: int = 0, jtf: int = 2, warmup=None, fill0=None,
          filler=None, level: int = 3, cw: int = 512, resident: bool = False,
          order: str = "s", dedupe: bool = True):
    """w: pos window width on tile jtf (= n_ref % JT); jtf = n_ref // JT.
    level (debug/timing): 0=matmuls only, 1=+exp (no accum), 2=+accum, 3=full.
    cw: rhs DMA chunk width (columns per dma_start; per-partition contiguous
    line = cw bytes).  resident: keep all of embT in SBUF across reps.
    order: 's' = bank-outer (one ldweights per matmul), 'dc' = contraction-
    outer (4 bank matmuls share one weight load; pair with dedupe=True).
    dedupe: strip redundant InstLdweights post-compile.
    nb: PSUM banks per exp tile (4 -> 2 psum bufs; 2 -> 4 psum bufs, PE can
    run further ahead of the Act stream).
    """
    if warmup is None:
        warmup = WARMUP
    if fill0 is None:
        fill0 = FILL0
    if filler is None:
        filler = FILLER
    key = (reps, w, jtf, warmup, fill0, filler, level, cw, resident, order,
           dedupe)
    if key in _build_cache:
        return _build_cache[key]

    nc = bacc.Bacc("TRN2", target_bir_lowering=False, debug=False)
    resT_d = nc.dram_tensor("resT", [D, RES], DT_MM, kind="ExternalInput")
    embT_d = nc.dram_tensor("embT", [D, N], DT_MM, kind="ExternalInput")
    partial_d = nc.dram_tensor("partial", [P, NOUT], F32, kind="ExternalOutput")

    # [D, X] viewed as [p, dc, x] with d = dc*128 + p
    resT = resT_d.ap().rearrange("(dc p) r -> p dc r", p=P)
    embT = embT_d.ap().rearrange("(dc p) n -> p dc n", p=P)

    with tile.TileContext(nc) as tc:
        with (
            tc.tile_pool(name="consts", bufs=1) as consts,
            tc.tile_pool(name="rhsp", bufs=8) as rhsp,
            tc.tile_pool(name="stats", bufs=2) as stats,
            tc.tile_pool(name="junk", bufs=2) as junkp,
            tc.tile_pool(name="psum", bufs=2, space=bass.MemorySpace.PSUM) as psum,
        ):
            # warmup/filler matmul operands (results never read; only the PE
            # busy-time matters).  Small so the memset is fast.
            wtile = consts.tile([P, 2, P], DT_MM)
            nc.vector.memset(wtile, 0.0)
            # warmup psum target: pool buffer 0 (reused by later real tiles)
            wps = psum.tile([P, NS, JS], F32, tag="ps")

            def dummy_mm(ps_target, n):
                for _ in range(n):
                    nc.tensor.matmul(
                        ps_target[:, 0:P],
                        wtile[:, 0:2, 0:P],
                        wtile[:, 0:2, 0:P],
                        start=True,
                        stop=True,
                        perf_mode=mybir.MatmulPerfMode.DoubleRow,
                        skip_group_check=True,
                    )

            # dummy activation at t=0: pulls the ACT_TABLE_LOAD for Exp off
            # the critical path (runs concurrently with the input DMAs)
            tjunk = junkp.tile([P, 1], F32, tag="tj")
            nc.scalar.activation(
                out=tjunk, in_=wtile[:, 0, 0:1],
                func=mybir.ActivationFunctionType.Exp,
            )

            dummy_mm(wps[:, 0, :], warmup)

            # resident stationary operand: this core's 512 embedding columns
            res = consts.tile([P, ND, RES], DT_MM)
            nc.sync.dma_start(out=res, in_=resT)

            if resident:
                embt_sb = consts.tile([P, ND, N], DT_MM)
                for s in range(N // cw):
                    nc.sync.dma_start(
                        out=embt_sb[:, :, s * cw : (s + 1) * cw],
                        in_=embT[:, :, s * cw : (s + 1) * cw],
                    )

            for rep in range(reps):
                st = stats.tile([P, NOUT], F32, tag="st")
                if level < 2:
                    nc.vector.memset(st, 0.0)

                for jt in range(NJT):
                    if resident:
                        chunks = [
                            embt_sb[:, :, jt * JT + s * JS : jt * JT + (s + 1) * JS]
                            for s in range(NS)
                        ]
                    else:
                        ntile = cw // JS     # slices per DMA tile
                        chunks = []
                        # spread descriptor generation (DIRECT2D, ~1us each)
                        # across sequencers so transfers start in parallel
                        engs = [nc.sync]
                        for t in range(JT // cw):
                            rhs = rhsp.tile([P, ND, cw], DT_MM, tag="rhs")
                            j0 = jt * JT + t * cw
                            eng = engs[(jt * (JT // cw) + t) % len(engs)]
                            eng.dma_start(out=rhs, in_=embT[:, :, j0 : j0 + cw])
                            chunks.extend(
                                rhs[:, :, u * JS : (u + 1) * JS] for u in range(ntile)
                            )
                    for ic in range(IC):
                        first = jt == 0 and ic == 0
                        last = jt == NJT - 1 and ic == IC - 1
                        split = first or last
                        ps = psum.tile([P, NS, JS], F32, tag="ps")
                        if filler and not first:
                            dummy_mm(ps[:, 0, :], filler)
                        if order == "dc" and not split:
                            # contraction-outer: 4 bank matmuls per weight
                            # load (redundant loads stripped by dedupe)
                            for dc2 in range(ND // 2):
                                for s in range(NS):
                                    nc.tensor.matmul(
                                        ps[:, s, :],
                                        res[:, 2 * dc2 : 2 * dc2 + 2, ic * P : (ic + 1) * P],
                                        chunks[s][:, 2 * dc2 : 2 * dc2 + 2, :],
                                        start=(dc2 == 0),
                                        stop=(dc2 == ND // 2 - 1),
                                        perf_mode=mybir.MatmulPerfMode.DoubleRow,
                                        skip_group_check=True,
                                    )
                            sloop = []
                        else:
                            sloop = list(range(NS))
                        for s in sloop:
                            for dc2 in range(ND // 2):
                                nc.tensor.matmul(
                                    ps[:, s, :],
                                    res[:, 2 * dc2 : 2 * dc2 + 2, ic * P : (ic + 1) * P],
                                    chunks[s][:, 2 * dc2 : 2 * dc2 + 2, :],
                                    start=(dc2 == 0),
                                    stop=(dc2 == ND // 2 - 1),
                                    perf_mode=mybir.MatmulPerfMode.DoubleRow,
                                )
                            if split and level >= 1:
                                # per-bank exp: starts the Act stream early
                                # (first tile) / shrinks the tail (last tile)
                                widx = IC * NJT + IC + (NS if last else 0) + s
                                nc.scalar.activation(
                                    out=ps[:, s, :],
                                    in_=ps[:, s, :],
                                    func=mybir.ActivationFunctionType.Exp,
                                    scale=SCALE,
                                    accum_out=(
                                        st[:, widx : widx + 1] if level >= 2 else None
                                    ),
                                )
                            if first and fill0 and s < NS - 1:
                                dummy_mm(wps[:, 0, :], fill0)
                        win_tile = level >= 3 and jt == jtf and w > 0
                        if not split and level >= 1:
                            # in-place exp on the 4-bank PSUM tile; accum_out
                            # emits the 2048-wide row sum for free.  Window
                            # tiles exp to SBUF instead so the DVE window
                            # pass doesn't extend the PSUM tile's lifetime.
                            ps_flat = ps.rearrange("p s j -> p (s j)")
                            idx = ic * NJT + jt
                            if win_tile:
                                ext = junkp.tile([P, JT], F32, tag="ext")
                                out_ap = ext
                            else:
                                out_ap = ps_flat
                            nc.scalar.activation(
                                out=out_ap,
                                in_=ps_flat,
                                func=mybir.ActivationFunctionType.Exp,
                                scale=SCALE,
                                accum_out=(
                                    st[:, idx : idx + 1] if level >= 2 else None
                                ),
                            )
                        if win_tile:
                            if split:
                                ext = ps.rearrange("p s j -> p (s j)")
                            wj = junkp.tile([P, w], F32, tag="wj")
                            widx = IC * NJT + ic
                            nc.vector.tensor_scalar(
                                out=wj,
                                in0=ext[:, 0:w],
                                scalar1=1.0,
                                scalar2=0.0,
                                op0=mybir.AluOpType.mult,
                                op1=mybir.AluOpType.add,
                                accum_out=st[:, widx : widx + 1],
                            )
                nc.sync.dma_start(out=partial_d.ap(), in_=st)

    nc.compile()
    if dedupe:
        dedupe_ldweights(nc)
    _build_cache[key] = nc
    return nc


def make_in_maps(embeddings: np.ndarray, labels: np.ndarray):
    emb = np.asarray(embeddings, dtype=np.float32)
    lab_f = np.asarray(labels).astype(np.float32)
    perm = np.argsort(-lab_f, kind="stable")
    emb_p = emb[perm]
    n_ref = int(lab_f.sum())
    assert n_ref <= NCORES * RES + 1024, "n_ref exceeds device+host capacity"
    embT = np.ascontiguousarray(emb_p.T).astype(F8NP)  # [D, N], one array
    in_maps = []
    for c in range(NCORES):
        in_maps.append(
            {
                "resT": np.ascontiguousarray(embT[:, c * RES : (c + 1) * RES]),
                "embT": embT,
            }
        )
    qf32 = embT.astype(np.float32)
    sq = np.einsum("dn,dn->n", qf32, qf32, dtype=np.float32)  # ||q(e_i)||^2
    ctx = {"emb_p": emb_p, "n_ref": n_ref, "sq": sq}
    return in_maps, ctx


def host_finish(partials, ctx):
    """partials: list of [P, NOUT] per core -> scalar loss (f32)."""
    n_ref = ctx["n_ref"]
    emb_p = ctx["emb_p"]
    sq = ctx["sq"]
    jtf = n_ref // JT
    dev_rows = NCORES * RES
    nb = IC * NJT + IC
    total = np.float64(0.0)
    for c in range(NCORES):
        arr = np.asarray(partials[c], np.float32)
        A = arr[:, : IC * NJT].reshape(P, IC, NJT).copy()
        A[:, 0, 0] = arr[:, nb : nb + NS].sum(axis=1)            # first tile
        A[:, IC - 1, NJT - 1] = arr[:, nb + NS : nb + 2 * NS].sum(axis=1)
        Wv = arr[:, IC * NJT : nb]                               # [P, IC]
        rows = c * RES + np.arange(IC)[None, :] * P + np.arange(P)[:, None]
        corr = np.exp(SCALE * sq[rows])                          # [P, IC]
        all_r = A.sum(axis=2) - corr
        pos_r = A[:, :, :jtf].sum(axis=2) + Wv - corr
        mask = rows < n_ref
        if not mask.any():
            continue
        contrib = np.where(
            mask,
            np.log(np.maximum(all_r, 1e-30) + EPS) - np.log(np.maximum(pos_r, 1e-30)),
            0.0,
        )
        total += contrib.sum(dtype=np.float64)
    if n_ref > dev_rows:
        hr = np.arange(dev_rows, n_ref)
        sim_h = (emb_p[hr] @ emb_p.T) * SCALE
        e_h = np.exp(sim_h)
        diag = e_h[np.arange(len(hr)), hr]
        all_h = e_h.sum(axis=1) - diag
        pos_h = e_h[:, :n_ref].sum(axis=1) - diag
        total += (np.log(all_h + EPS) - np.log(pos_h)).sum(dtype=np.float64)
    loss = total / max(n_ref, 1)
    return np.float32(loss)


def kernel(embeddings: np.ndarray, labels: np.ndarray) -> np.ndarray:
    lab_f = np.asarray(labels).astype(np.float32)
    n_ref = float(lab_f.sum())
    if n_ref < 2:
        return np.float32(0.0)

    in_maps, ctx = make_in_maps(embeddings, labels)
    w = ctx["n_ref"] % JT
    jtf = ctx["n_ref"] // JT
    nc = build(reps=1, w=w, jtf=jtf)
    res = run_bass_kernel_spmd(nc, in_maps, core_ids=list(range(NCORES)))
    partials = [res.results[c]["partial"] for c in range(NCORES)]
    return np.asarray(host_finish(partials, ctx), dtype=np.float32)


# revision 36
# speedup vs baseline: 1.6425x; 1.2174x over previous
"""Contrastive loss (SupCon-style) on 8 Trainium2 NeuronCores — v2.

Reference (N=8192, D=1024, T=0.1, normalized embeddings):
    sim = emb @ emb.T / T;  e = exp(sim)
    all_sum_i = sum_j e_ij - e_ii
    pos_sum_i = sum_j e_ij * lab_j - e_ii * lab_i
    loss = mean_{i: lab_i=1} [ log(all_sum_i + eps) - log(pos_sum_i) ]

Structure (v2):
  * Host permutes embeddings ref-first (one global permutation, identical for
    every core) and quantizes once to fp8: embT [D, N].  Core c's stationary
    operand is its 512-row slice resT = embT[:, c*512:(c+1)*512].
  * The diagonal e_ii is NOT removed on device.  The device accumulates plain
    per-tile row sums; the host subtracts exp(10 * ||q(e_i)||^2) computed
    from the same fp8 values (fp8 products are exact in fp32; only the
    reduction order differs).
  * pos_sum needs no label mask on device: ref columns are a global prefix,
    so pos = sum of the first JT_FULL_POS whole-tile sums + one narrow
    window pass of width W = n_ref mod 2048 on tile JT_FULL_POS (DVE).
  * Device rows = 8*512 = 4096; leftover ref rows (n_ref - 4096) are
    computed on the host in f32 (tiny GEMM).  Final log/mask/reduce on host.
  * Device returns [P, NOUT] f32 of raw sums per core.

Per (ic, jt): 16 fp8 DoubleRow matmuls -> 4-bank PSUM tile; ScalarE
exp(10*x) IN-PLACE on PSUM with accum_out emitting the tile row-sum.  The
first and last tiles run exp per 512-wide PSUM bank instead (4 partial sums
each) so the Act stream starts ~4us earlier and the tail exp shrinks.
Dummy warmup matmuls at t=0 ramp the PE p-state while the first DMAs land;
fill0 dummies bridge the jt0 chunk-DMA gaps; a few filler dummies per tile
bridge the Act/PE pace gap.  All DMAs ride the sync queue (HWDGE).
"""

import numpy as np

import concourse.bass as bass
import concourse.tile as tile
import concourse.mybir as mybir
from concourse import bacc
from concourse.bass_utils import run_bass_kernel_spmd

N, D = 8192, 1024
NCORES = 8
P = 128             # partitions
RES = 512           # per-core device rows (4 chunks of 128)
IC = RES // P       # 4 row chunks per core
JS = 512            # j-slice width (one PSUM bank of fp32)
NS = 4              # PSUM banks per tile -> 2048-wide exp instrs
JT = NS * JS        # j-tile width (2048)
NJT = N // JT       # 4 j tiles
ND = D // P         # 8 contraction chunks
SCALE = 10.0        # 1 / TEMPERATURE
EPS = 1e-8

F32 = mybir.dt.float32
DT_MM = mybir.dt.float8e4
F8NP = mybir.dt.np(DT_MM)

# HW-measured: steady-state dummy matmuls are a net loss (each pays the
# full 256-column DoubleRow LDWEIGHTS, ~213ns) — but a warmup train during
# the initial DMA wait warms the PE throttle state for ~-4us.  With dedupe
# the warmup dummies share one weight load.
WARMUP = 56         # dummy matmuls at t=0 (overlap the first DMAs)
FILL0 = 0           # dummies between (jt0, ic0) chunk groups
FILLER = 0          # dummies per (ic, jt)

# st column layout: [0:16) whole-tile row sums (idx = ic*NJT + jt; unused for
# the two bank-split tiles), [16:20) window sums, [20:24) first-tile bank
# sums, [24:28) last-tile bank sums
NOUT = IC * NJT + IC + 2 * NS

# Triangle mode: core c owns row-chunks {c, 8+c, 16+c, 24+c} (one per
# 1024-row band); the band-b chunk skips columns < 1024*b (lower triangle at
# 512-col granularity, SPMD-uniform).  The skipped e_xj are recovered on the
# host from column sums of the transposed tiles, which 4 dump tiles per core
# provide: (ic,jt) in DUMPS, exp'd to SBUF bf16 and DMA'd out.
DUMPS = ((0, 0), (0, 1), (1, 1), (2, 1))


def _skip(ic, jt, s):
    """slice (jt*NS + s) of row-band ic is below the diagonal band."""
    return jt * NS + s < 2 * ic

_build_cache = {}


def dedupe_ldweights(nc):
    """Remove InstLdweights that reload the exact weights already resident
    in the PE array (same source AP + perf mode as the previous load),
    merging their waits/updates into the next PE instruction."""
    PE = mybir.EngineType.PE
    total = 0
    for b in nc.main_func.blocks:
        keep = []
        last_sig = None
        pend_w, pend_u = [], []
        removed = 0
        for i in b.instructions:
            if isinstance(i, mybir.InstLdweights):
                sig = (str(i.ins[0]), str(i.perf_mode))
                if sig == last_sig:
                    si = i.sync_info
                    if si is not None:
                        pend_w += list(si.on_wait)
                        pend_u += list(si.on_update)
                    removed += 1
                    continue
                last_sig = sig
            if (pend_w or pend_u) and getattr(i, "engine", None) == PE:
                si = i.sync_info
                if si is None:
                    i.sync_info = mybir.SyncInfo(on_wait=pend_w, on_update=pend_u)
                else:
                    si.on_wait = list(si.on_wait) + pend_w
                    si.on_update = list(si.on_update) + pend_u
                pend_w, pend_u = [], []
            keep.append(i)
        if removed:
            assert not pend_w and not pend_u, "dangling sync from removed ldw"
            n_old = len(b.instructions)
            for _ in range(n_old):
                b.instructions.pop()
            for i in keep:
                b.instructions.append(i)
            total += removed
    return total


def build(reps: int = 1, w: int = 0, jtf: int = 2, warmup=None, fill0=None,
          filler=None, level: int = 3, cw: int = 512, resident: bool = False,
          order: str = "s", dedupe: bool = True, tri: bool = False):
    """w: pos window width on tile jtf (= n_ref % JT); jtf = n_ref // JT.
    level (debug/timing): 0=matmuls only, 1=+exp (no accum), 2=+accum, 3=full.
    cw: rhs DMA chunk width (columns per dma_start; per-partition contiguous
    line = cw bytes).  resident: keep all of embT in SBUF across reps.
    order: 's' = bank-outer (one ldweights per matmul), 'dc' = contraction-
    outer (4 bank matmuls share one weight load; pair with dedupe=True).
    dedupe: strip redundant InstLdweights post-compile.
    tri: skip below-diagonal-band slices and dump transpose-source tiles.
    """
    if warmup is None:
        warmup = WARMUP
    if fill0 is None:
        fill0 = FILL0
    if filler is None:
        filler = FILLER
    key = (reps, w, jtf, warmup, fill0, filler, level, cw, resident, order,
           dedupe, tri)
    if key in _build_cache:
        return _build_cache[key]

    nc = bacc.Bacc("TRN2", target_bir_lowering=False, debug=False)
    resT_d = nc.dram_tensor("resT", [D, RES], DT_MM, kind="ExternalInput")
    embT_d = nc.dram_tensor("embT", [D, N], DT_MM, kind="ExternalInput")
    partial_d = nc.dram_tensor("partial", [P, NOUT], F32, kind="ExternalOutput")
    BF16 = mybir.dt.bfloat16
    extd_d = None
    if tri:
        extd_d = nc.dram_tensor(
            "extd", [P, len(DUMPS) * JT], BF16, kind="ExternalOutput"
        )

    # [D, X] viewed as [p, dc, x] with d = dc*128 + p
    resT = resT_d.ap().rearrange("(dc p) r -> p dc r", p=P)
    embT = embT_d.ap().rearrange("(dc p) n -> p dc n", p=P)

    with tile.TileContext(nc) as tc:
        with (
            tc.tile_pool(name="consts", bufs=1) as consts,
            tc.tile_pool(name="rhsp", bufs=8) as rhsp,
            tc.tile_pool(name="stats", bufs=2) as stats,
            tc.tile_pool(name="junk", bufs=2) as junkp,
            tc.tile_pool(name="psum", bufs=2, space=bass.MemorySpace.PSUM) as psum,
        ):
            # warmup/filler matmul operands (results never read; only the PE
            # busy-time matters).  Small so the memset is fast.
            wtile = consts.tile([P, 2, P], DT_MM)
            nc.vector.memset(wtile, 0.0)
            # warmup psum target: pool buffer 0 (reused by later real tiles)
            wps = psum.tile([P, NS, JS], F32, tag="ps")

            def dummy_mm(ps_target, n):
                for _ in range(n):
                    nc.tensor.matmul(
                        ps_target[:, 0:P],
                        wtile[:, 0:2, 0:P],
                        wtile[:, 0:2, 0:P],
                        start=True,
                        stop=True,
                        perf_mode=mybir.MatmulPerfMode.DoubleRow,
                        skip_group_check=True,
                    )

            # dummy activation at t=0: pulls the ACT_TABLE_LOAD for Exp off
            # the critical path (runs concurrently with the input DMAs)
            tjunk = junkp.tile([P, 1], F32, tag="tj")
            nc.scalar.activation(
                out=tjunk, in_=wtile[:, 0, 0:1],
                func=mybir.ActivationFunctionType.Exp,
            )

            dummy_mm(wps[:, 0, :], warmup)

            # resident stationary operand: this core's 512 embedding columns
            res = consts.tile([P, ND, RES], DT_MM)
            nc.sync.dma_start(out=res, in_=resT)

            if resident:
                embt_sb = consts.tile([P, ND, N], DT_MM)
                for s in range(N // cw):
                    nc.sync.dma_start(
                        out=embt_sb[:, :, s * cw : (s + 1) * cw],
                        in_=embT[:, :, s * cw : (s + 1) * cw],
                    )

            for rep in range(reps):
                st = stats.tile([P, NOUT], F32, tag="st")
                if level < 2:
                    nc.vector.memset(st, 0.0)

                # triangle mode: process dense (high) j-tiles first so the
                # DMA-paced start feeds a full PE stream; the sparse low
                # tiles run last against prefetched chunks
                jts = list(range(NJT - 1, -1, -1)) if tri else list(range(NJT))
                for jt in jts:
                    if resident:
                        chunks = [
                            embt_sb[:, :, jt * JT + s * JS : jt * JT + (s + 1) * JS]
                            for s in range(NS)
                        ]
                    else:
                        ntile = cw // JS     # slices per DMA tile
                        chunks = []
                        # spread descriptor generation (DIRECT2D, ~1us each)
                        # across sequencers so transfers start in parallel
                        engs = [nc.sync]
                        for t in range(JT // cw):
                            rhs = rhsp.tile([P, ND, cw], DT_MM, tag="rhs")
                            j0 = jt * JT + t * cw
                            eng = engs[(jt * (JT // cw) + t) % len(engs)]
                            eng.dma_start(out=rhs, in_=embT[:, :, j0 : j0 + cw])
                            chunks.extend(
                                rhs[:, :, u * JS : (u + 1) * JS] for u in range(ntile)
                            )
                    for ic in range(IC):
                        first = jt == jts[0] and ic == 0
                        last = (not tri) and jt == NJT - 1 and ic == IC - 1
                        split = first or last
                        kept = [
                            s for s in range(NS)
                            if not (tri and _skip(ic, jt, s))
                        ]
                        if not kept:
                            continue
                        s0 = kept[0]
                        is_dump = tri and (ic, jt) in DUMPS
                        ext_b = None
                        if is_dump:
                            ext_b = junkp.tile([P, JT], BF16, tag="extb")
                        ps = psum.tile([P, NS, JS], F32, tag="ps")
                        if filler and not first:
                            dummy_mm(ps[:, 0, :], filler)
                        if order == "dc" and not split and not tri:
                            # contraction-outer: 4 bank matmuls per weight
                            # load (redundant loads stripped by dedupe)
                            for dc2 in range(ND // 2):
                                for s in range(NS):
                                    nc.tensor.matmul(
                                        ps[:, s, :],
                                        res[:, 2 * dc2 : 2 * dc2 + 2, ic * P : (ic + 1) * P],
                                        chunks[s][:, 2 * dc2 : 2 * dc2 + 2, :],
                                        start=(dc2 == 0),
                                        stop=(dc2 == ND // 2 - 1),
                                        perf_mode=mybir.MatmulPerfMode.DoubleRow,
                                        skip_group_check=True,
                                    )
                            sloop = []
                        else:
                            sloop = kept
                        for s in sloop:
                            for dc2 in range(ND // 2):
                                nc.tensor.matmul(
                                    ps[:, s, :],
                                    res[:, 2 * dc2 : 2 * dc2 + 2, ic * P : (ic + 1) * P],
                                    chunks[s][:, 2 * dc2 : 2 * dc2 + 2, :],
                                    start=(dc2 == 0),
                                    stop=(dc2 == ND // 2 - 1),
                                    perf_mode=mybir.MatmulPerfMode.DoubleRow,
                                )
                            if split and level >= 1:
                                # per-bank exp: starts the Act stream early
                                # (first tile) / shrinks the tail (last tile)
                                widx = IC * NJT + IC + (NS if last else 0) + s
                                bout = (
                                    ext_b[:, s * JS : (s + 1) * JS]
                                    if is_dump
                                    else ps[:, s, :]
                                )
                                nc.scalar.activation(
                                    out=bout,
                                    in_=ps[:, s, :],
                                    func=mybir.ActivationFunctionType.Exp,
                                    scale=SCALE,
                                    accum_out=(
                                        st[:, widx : widx + 1] if level >= 2 else None
                                    ),
                                )
                            if first and fill0 and s < NS - 1:
                                dummy_mm(wps[:, 0, :], fill0)
                        win_tile = level >= 3 and jt == jtf and w > 0
                        if not split and level >= 1:
                            # in-place exp on the kept PSUM banks; accum_out
                            # emits the row sum for free.  Window tiles exp
                            # to SBUF f32 (DVE reads it off the PSUM path);
                            # dump tiles exp to SBUF bf16 (DMA'd to HBM for
                            # host-side transpose recovery).
                            ps_flat = ps[:, s0:NS, :].rearrange("p s j -> p (s j)")
                            idx = ic * NJT + jt
                            if is_dump:
                                out_ap = ext_b
                            elif win_tile:
                                ext = junkp.tile([P, JT], F32, tag="ext")
                                out_ap = ext
                            else:
                                out_ap = ps_flat
                            nc.scalar.activation(
                                out=out_ap,
                                in_=ps_flat,
                                func=mybir.ActivationFunctionType.Exp,
                                scale=SCALE,
                                accum_out=(
                                    st[:, idx : idx + 1] if level >= 2 else None
                                ),
                            )
                        if is_dump and level >= 1:
                            q = DUMPS.index((ic, jt))
                            nc.sync.dma_start(
                                out=extd_d.ap()[:, q * JT : (q + 1) * JT],
                                in_=ext_b,
                            )
                        if win_tile:
                            if split:
                                ext = ps.rearrange("p s j -> p (s j)")
                            wj = junkp.tile([P, w], F32, tag="wj")
                            widx = IC * NJT + ic
                            nc.vector.tensor_scalar(
                                out=wj,
                                in0=ext[:, 0:w],
                                scalar1=1.0,
                                scalar2=0.0,
                                op0=mybir.AluOpType.mult,
                                op1=mybir.AluOpType.add,
                                accum_out=st[:, widx : widx + 1],
                            )
                nc.sync.dma_start(out=partial_d.ap(), in_=st)

    nc.compile()
    if dedupe:
        dedupe_ldweights(nc)
    _build_cache[key] = nc
    return nc


def make_in_maps(embeddings: np.ndarray, labels: np.ndarray, tri=None):
    emb = np.asarray(embeddings, dtype=np.float32)
    lab_f = np.asarray(labels).astype(np.float32)
    perm = np.argsort(-lab_f, kind="stable")
    emb_p = emb[perm]
    n_ref = int(lab_f.sum())
    assert n_ref <= NCORES * RES + 1024, "n_ref exceeds device+host capacity"
    if tri is None:
        tri = n_ref >= NCORES * RES  # triangle math assumes all device rows ref
    embT = np.ascontiguousarray(emb_p.T).astype(F8NP)  # [D, N], one array
    in_maps = []
    for c in range(NCORES):
        if tri:
            # core c owns row-chunks {c, 8+c, 16+c, 24+c} (one per band)
            resT = np.concatenate(
                [
                    embT[:, (8 * ic + c) * P : (8 * ic + c + 1) * P]
                    for ic in range(IC)
                ],
                axis=1,
            )
        else:
            resT = embT[:, c * RES : (c + 1) * RES]
        in_maps.append(
            {
                "resT": np.ascontiguousarray(resT),
                "embT": embT,
            }
        )
    qf32 = embT.astype(np.float32)
    sq = np.einsum("dn,dn->n", qf32, qf32, dtype=np.float32)  # ||q(e_i)||^2
    ctx = {"emb_p": emb_p, "n_ref": n_ref, "sq": sq, "tri": tri}
    return in_maps, ctx


def host_finish(partials, ctx, extds=None):
    """partials: list of [P, NOUT] per core -> scalar loss (f32).
    extds: list of [P, 4*JT] bf16 dump tiles per core (triangle mode)."""
    n_ref = ctx["n_ref"]
    emb_p = ctx["emb_p"]
    sq = ctx["sq"]
    tri = ctx.get("tri", False)
    jtf = n_ref // JT
    dev_rows = NCORES * RES
    nb = IC * NJT + IC
    total = np.float64(0.0)
    missing = None
    if tri:
        # CS[band J, col x] = sum over band-J rows of e_{row, x}, summed
        # across cores; dump quarter q of core c covers (ic, jt) = DUMPS[q]
        CS = np.zeros((IC - 1, dev_rows), np.float64)
        for c in range(NCORES):
            ed = np.asarray(extds[c], np.float32)  # [P, 4*JT]
            for q, (ic, jt) in enumerate(DUMPS):
                cs = ed[:, q * JT : (q + 1) * JT].sum(axis=0)  # [JT]
                CS[ic, jt * JT : (jt + 1) * JT] += cs
        # missing(x) = sum_{J < band(x)} CS[J, x]
        band = np.arange(dev_rows) // (8 * P)
        missing = np.zeros(dev_rows, np.float64)
        for J in range(IC - 1):
            missing += np.where(band > J, CS[J], 0.0)
    for c in range(NCORES):
        arr = np.asarray(partials[c], np.float32)
        A = arr[:, : IC * NJT].reshape(P, IC, NJT).copy()
        if tri:
            # first processed tile is (ic0, jt3); no last-tile split
            A[:, 0, NJT - 1] = arr[:, nb : nb + NS].sum(axis=1)
        else:
            A[:, 0, 0] = arr[:, nb : nb + NS].sum(axis=1)        # first tile
            A[:, IC - 1, NJT - 1] = arr[:, nb + NS : nb + 2 * NS].sum(axis=1)
        Wv = arr[:, IC * NJT : nb]                               # [P, IC]
        if tri:
            # fully-skipped tiles' st columns are garbage -> zero them
            for ic in range(IC):
                for jt in range(NJT):
                    if all(_skip(ic, jt, s) for s in range(NS)):
                        A[:, ic, jt] = 0.0
            rows = (8 * np.arange(IC)[None, :] + c) * P + np.arange(P)[:, None]
        else:
            rows = c * RES + np.arange(IC)[None, :] * P + np.arange(P)[:, None]
        corr = np.exp(SCALE * sq[rows])                          # [P, IC]
        all_r = A.sum(axis=2) - corr
        pos_r = A[:, :, :jtf].sum(axis=2) + Wv - corr
        if tri:
            m = missing[rows].astype(np.float32)
            all_r = all_r + m
            pos_r = pos_r + m
        mask = rows < n_ref
        if not mask.any():
            continue
        contrib = np.where(
            mask,
            np.log(np.maximum(all_r, 1e-30) + EPS) - np.log(np.maximum(pos_r, 1e-30)),
            0.0,
        )
        total += contrib.sum(dtype=np.float64)
    if n_ref > dev_rows:
        hr = np.arange(dev_rows, n_ref)
        sim_h = (emb_p[hr] @ emb_p.T) * SCALE
        e_h = np.exp(sim_h)
        diag = e_h[np.arange(len(hr)), hr]
        all_h = e_h.sum(axis=1) - diag
        pos_h = e_h[:, :n_ref].sum(axis=1) - diag
        total += (np.log(all_h + EPS) - np.log(pos_h)).sum(dtype=np.float64)
    loss = total / max(n_ref, 1)
    return np.float32(loss)


def kernel(embeddings: np.ndarray, labels: np.ndarray) -> np.ndarray:
    lab_f = np.asarray(labels).astype(np.float32)
    n_ref = float(lab_f.sum())
    if n_ref < 2:
        return np.float32(0.0)

    in_maps, ctx = make_in_maps(embeddings, labels)
    w = ctx["n_ref"] % JT
    jtf = ctx["n_ref"] // JT
    nc = build(reps=1, w=w, jtf=jtf, tri=ctx["tri"])
    res = run_bass_kernel_spmd(nc, in_maps, core_ids=list(range(NCORES)))
    partials = [res.results[c]["partial"] for c in range(NCORES)]
    extds = (
        [res.results[c]["extd"] for c in range(NCORES)] if ctx["tri"] else None
    )
    return np.asarray(host_finish(partials, ctx, extds), dtype=np.float32)


# revision 40
# speedup vs baseline: 1.6674x; 1.0151x over previous
"""Contrastive loss (SupCon-style) on 8 Trainium2 NeuronCores — v2.

Reference (N=8192, D=1024, T=0.1, normalized embeddings):
    sim = emb @ emb.T / T;  e = exp(sim)
    all_sum_i = sum_j e_ij - e_ii
    pos_sum_i = sum_j e_ij * lab_j - e_ii * lab_i
    loss = mean_{i: lab_i=1} [ log(all_sum_i + eps) - log(pos_sum_i) ]

Structure (v2):
  * Host permutes embeddings ref-first (one global permutation, identical for
    every core) and quantizes once to fp8: embT [D, N].  Core c's stationary
    operand is its 512-row slice resT = embT[:, c*512:(c+1)*512].
  * The diagonal e_ii is NOT removed on device.  The device accumulates plain
    per-tile row sums; the host subtracts exp(10 * ||q(e_i)||^2) computed
    from the same fp8 values (fp8 products are exact in fp32; only the
    reduction order differs).
  * pos_sum needs no label mask on device: ref columns are a global prefix,
    so pos = sum of the first JT_FULL_POS whole-tile sums + one narrow
    window pass of width W = n_ref mod 2048 on tile JT_FULL_POS (DVE).
  * Device rows = 8*512 = 4096; leftover ref rows (n_ref - 4096) are
    computed on the host in f32 (tiny GEMM).  Final log/mask/reduce on host.
  * Device returns [P, NOUT] f32 of raw sums per core.

Per (ic, jt): 16 fp8 DoubleRow matmuls -> 4-bank PSUM tile; ScalarE
exp(10*x) IN-PLACE on PSUM with accum_out emitting the tile row-sum.  The
first and last tiles run exp per 512-wide PSUM bank instead (4 partial sums
each) so the Act stream starts ~4us earlier and the tail exp shrinks.
Dummy warmup matmuls at t=0 ramp the PE p-state while the first DMAs land;
fill0 dummies bridge the jt0 chunk-DMA gaps; a few filler dummies per tile
bridge the Act/PE pace gap.  All DMAs ride the sync queue (HWDGE).
"""

import numpy as np

import concourse.bass as bass
import concourse.tile as tile
import concourse.mybir as mybir
from concourse import bacc
from concourse.bass_utils import run_bass_kernel_spmd

N, D = 8192, 1024
NCORES = 8
P = 128             # partitions
RES = 512           # per-core device rows (4 chunks of 128)
IC = RES // P       # 4 row chunks per core
JS = 512            # j-slice width (one PSUM bank of fp32)
NS = 4              # PSUM banks per tile -> 2048-wide exp instrs
JT = NS * JS        # j-tile width (2048)
NJT = N // JT       # 4 j tiles
ND = D // P         # 8 contraction chunks
SCALE = 10.0        # 1 / TEMPERATURE
EPS = 1e-8

F32 = mybir.dt.float32
DT_MM = mybir.dt.float8e4
F8NP = mybir.dt.np(DT_MM)

# HW-measured: steady-state dummy matmuls are a net loss (each pays the
# full 256-column DoubleRow LDWEIGHTS, ~213ns) — but a warmup train during
# the initial DMA wait warms the PE throttle state for ~-4us.  With dedupe
# the warmup dummies share one weight load.
WARMUP = 56         # dummy matmuls at t=0 (overlap the first DMAs)
FILL0 = 0           # dummies between (jt0, ic0) chunk groups
FILLER = 0          # dummies per (ic, jt)

# st column layout: [0:16) whole-tile row sums (idx = ic*NJT + jt; unused for
# the two bank-split tiles), [16:20) window sums, [20:24) first-tile bank
# sums, [24:28) last-tile bank sums
NOUT = IC * NJT + IC + 2 * NS

# Triangle mode: core c owns row-chunks {c, 8+c, 16+c, 24+c} (one per
# 1024-row band); the band-b chunk skips columns < 1024*b (lower triangle at
# 512-col granularity, SPMD-uniform).  The skipped e_xj are recovered on the
# host from column sums of the transposed tiles, which 4 dump tiles per core
# provide: (ic,jt) in DUMPS, exp'd to SBUF bf16 and DMA'd out.
# (ic, jt, lo, hi): only cols [lo, hi) of the tile are read by the host
# (CS[0] is used for x >= 1024 and CS[2] for x >= 3072 only)
DUMPS = ((0, 0, JS * 2, JT), (0, 1, 0, JT), (1, 1, 0, JT), (2, 1, JS * 2, JT))


def _skip(ic, jt, s):
    """slice (jt*NS + s) of row-band ic is below the diagonal band."""
    return jt * NS + s < 2 * ic

_build_cache = {}


def dedupe_ldweights(nc):
    """Remove InstLdweights that reload the exact weights already resident
    in the PE array (same source AP + perf mode as the previous load),
    merging their waits/updates into the next PE instruction."""
    PE = mybir.EngineType.PE
    total = 0
    for b in nc.main_func.blocks:
        keep = []
        last_sig = None
        pend_w, pend_u = [], []
        removed = 0
        for i in b.instructions:
            if isinstance(i, mybir.InstLdweights):
                sig = (str(i.ins[0]), str(i.perf_mode))
                if sig == last_sig:
                    si = i.sync_info
                    if si is not None:
                        pend_w += list(si.on_wait)
                        pend_u += list(si.on_update)
                    removed += 1
                    continue
                last_sig = sig
            if (pend_w or pend_u) and getattr(i, "engine", None) == PE:
                si = i.sync_info
                if si is None:
                    i.sync_info = mybir.SyncInfo(on_wait=pend_w, on_update=pend_u)
                else:
                    si.on_wait = list(si.on_wait) + pend_w
                    si.on_update = list(si.on_update) + pend_u
                pend_w, pend_u = [], []
            keep.append(i)
        if removed:
            assert not pend_w and not pend_u, "dangling sync from removed ldw"
            n_old = len(b.instructions)
            for _ in range(n_old):
                b.instructions.pop()
            for i in keep:
                b.instructions.append(i)
            total += removed
    return total


def build(reps: int = 1, w: int = 0, jtf: int = 2, warmup=None, fill0=None,
          filler=None, level: int = 3, cw: int = 512, resident: bool = False,
          order: str = "s", dedupe: bool = True, tri: bool = False):
    """w: pos window width on tile jtf (= n_ref % JT); jtf = n_ref // JT.
    level (debug/timing): 0=matmuls only, 1=+exp (no accum), 2=+accum, 3=full.
    cw: rhs DMA chunk width (columns per dma_start; per-partition contiguous
    line = cw bytes).  resident: keep all of embT in SBUF across reps.
    order: 's' = bank-outer (one ldweights per matmul), 'dc' = contraction-
    outer (4 bank matmuls share one weight load; pair with dedupe=True).
    dedupe: strip redundant InstLdweights post-compile.
    tri: skip below-diagonal-band slices and dump transpose-source tiles.
    """
    if warmup is None:
        warmup = WARMUP
    if fill0 is None:
        fill0 = FILL0
    if filler is None:
        filler = FILLER
    key = (reps, w, jtf, warmup, fill0, filler, level, cw, resident, order,
           dedupe, tri)
    if key in _build_cache:
        return _build_cache[key]

    nc = bacc.Bacc("TRN2", target_bir_lowering=False, debug=False)
    resT_d = nc.dram_tensor("resT", [D, RES], DT_MM, kind="ExternalInput")
    embT_d = nc.dram_tensor("embT", [D, N], DT_MM, kind="ExternalInput")
    partial_d = nc.dram_tensor("partial", [P, NOUT], F32, kind="ExternalOutput")
    BF16 = mybir.dt.bfloat16
    extd_d = None
    if tri:
        extd_d = nc.dram_tensor(
            "extd", [P, len(DUMPS) * JT], BF16, kind="ExternalOutput"
        )

    # [D, X] viewed as [p, dc, x] with d = dc*128 + p
    resT = resT_d.ap().rearrange("(dc p) r -> p dc r", p=P)
    embT = embT_d.ap().rearrange("(dc p) n -> p dc n", p=P)

    with tile.TileContext(nc) as tc:
        with (
            tc.tile_pool(name="consts", bufs=1) as consts,
            tc.tile_pool(name="rhsp", bufs=8) as rhsp,
            tc.tile_pool(name="stats", bufs=2) as stats,
            tc.tile_pool(name="junk", bufs=2) as junkp,
            tc.tile_pool(name="psum", bufs=2, space=bass.MemorySpace.PSUM) as psum,
        ):
            # warmup/filler matmul operands (results never read; only the PE
            # busy-time matters).  Small so the memset is fast.
            wtile = consts.tile([P, 2, P], DT_MM)
            nc.vector.memset(wtile, 0.0)
            # warmup psum target: pool buffer 0 (reused by later real tiles)
            wps = psum.tile([P, NS, JS], F32, tag="ps")

            def dummy_mm(ps_target, n):
                for _ in range(n):
                    nc.tensor.matmul(
                        ps_target[:, 0:P],
                        wtile[:, 0:2, 0:P],
                        wtile[:, 0:2, 0:P],
                        start=True,
                        stop=True,
                        perf_mode=mybir.MatmulPerfMode.DoubleRow,
                        skip_group_check=True,
                    )

            # dummy activation at t=0: pulls the ACT_TABLE_LOAD for Exp off
            # the critical path (runs concurrently with the input DMAs)
            tjunk = junkp.tile([P, 1], F32, tag="tj")
            nc.scalar.activation(
                out=tjunk, in_=wtile[:, 0, 0:1],
                func=mybir.ActivationFunctionType.Exp,
            )

            dummy_mm(wps[:, 0, :], warmup)

            # resident stationary operand: this core's 512 embedding columns
            res = consts.tile([P, ND, RES], DT_MM)
            nc.sync.dma_start(out=res, in_=resT)

            if resident:
                embt_sb = consts.tile([P, ND, N], DT_MM)
                for s in range(N // cw):
                    nc.sync.dma_start(
                        out=embt_sb[:, :, s * cw : (s + 1) * cw],
                        in_=embT[:, :, s * cw : (s + 1) * cw],
                    )

            for rep in range(reps):
                st = stats.tile([P, NOUT], F32, tag="st")
                if level < 2:
                    nc.vector.memset(st, 0.0)

                # triangle mode: process dense (high) j-tiles first so the
                # DMA-paced start feeds a full PE stream; the sparse low
                # tiles run last against prefetched chunks
                jts = list(range(NJT - 1, -1, -1)) if tri else list(range(NJT))
                for jt in jts:
                    if resident:
                        chunks = [
                            embt_sb[:, :, jt * JT + s * JS : jt * JT + (s + 1) * JS]
                            for s in range(NS)
                        ]
                    else:
                        ntile = cw // JS     # slices per DMA tile
                        chunks = []
                        # spread descriptor generation (DIRECT2D, ~1us each)
                        # across sequencers so transfers start in parallel
                        engs = [nc.sync]
                        for t in range(JT // cw):
                            rhs = rhsp.tile([P, ND, cw], DT_MM, tag="rhs")
                            j0 = jt * JT + t * cw
                            eng = engs[(jt * (JT // cw) + t) % len(engs)]
                            eng.dma_start(out=rhs, in_=embT[:, :, j0 : j0 + cw])
                            chunks.extend(
                                rhs[:, :, u * JS : (u + 1) * JS] for u in range(ntile)
                            )
                    for ic in range(IC):
                        first = jt == jts[0] and ic == 0
                        last = (not tri) and jt == NJT - 1 and ic == IC - 1
                        split = first or last
                        kept = [
                            s for s in range(NS)
                            if not (tri and _skip(ic, jt, s))
                        ]
                        if not kept:
                            continue
                        s0 = kept[0]
                        dump_q = next(
                            (
                                qi for qi, d in enumerate(DUMPS)
                                if d[0] == ic and d[1] == jt
                            ),
                            None,
                        ) if tri else None
                        is_dump = dump_q is not None
                        ext_b = None
                        if is_dump:
                            ext_b = junkp.tile([P, JT], BF16, tag="extb")
                        ps = psum.tile([P, NS, JS], F32, tag="ps")
                        if filler and not first:
                            dummy_mm(ps[:, 0, :], filler)
                        if order == "dc" and not split and not tri:
                            # contraction-outer: 4 bank matmuls per weight
                            # load (redundant loads stripped by dedupe)
                            for dc2 in range(ND // 2):
                                for s in range(NS):
                                    nc.tensor.matmul(
                                        ps[:, s, :],
                                        res[:, 2 * dc2 : 2 * dc2 + 2, ic * P : (ic + 1) * P],
                                        chunks[s][:, 2 * dc2 : 2 * dc2 + 2, :],
                                        start=(dc2 == 0),
                                        stop=(dc2 == ND // 2 - 1),
                                        perf_mode=mybir.MatmulPerfMode.DoubleRow,
                                        skip_group_check=True,
                                    )
                            sloop = []
                        else:
                            sloop = kept
                        for s in sloop:
                            for dc2 in range(ND // 2):
                                nc.tensor.matmul(
                                    ps[:, s, :],
                                    res[:, 2 * dc2 : 2 * dc2 + 2, ic * P : (ic + 1) * P],
                                    chunks[s][:, 2 * dc2 : 2 * dc2 + 2, :],
                                    start=(dc2 == 0),
                                    stop=(dc2 == ND // 2 - 1),
                                    perf_mode=mybir.MatmulPerfMode.DoubleRow,
                                )
                            if split and level >= 1:
                                # per-bank exp: starts the Act stream early
                                # (first tile) / shrinks the tail (last tile)
                                widx = IC * NJT + IC + (NS if last else 0) + s
                                bout = (
                                    ext_b[:, s * JS : (s + 1) * JS]
                                    if is_dump
                                    else ps[:, s, :]
                                )
                                nc.scalar.activation(
                                    out=bout,
                                    in_=ps[:, s, :],
                                    func=mybir.ActivationFunctionType.Exp,
                                    scale=SCALE,
                                    accum_out=(
                                        st[:, widx : widx + 1] if level >= 2 else None
                                    ),
                                )
                            if first and fill0 and s < NS - 1:
                                dummy_mm(wps[:, 0, :], fill0)
                        win_tile = level >= 3 and jt == jtf and w > 0
                        if not split and level >= 1:
                            # in-place exp on the kept PSUM banks; accum_out
                            # emits the row sum for free.  Window tiles exp
                            # to SBUF f32 (DVE reads it off the PSUM path);
                            # dump tiles exp to SBUF bf16 (DMA'd to HBM for
                            # host-side transpose recovery).
                            ps_flat = ps[:, s0:NS, :].rearrange("p s j -> p (s j)")
                            idx = ic * NJT + jt
                            if is_dump:
                                out_ap = ext_b
                            elif win_tile:
                                ext = junkp.tile([P, JT], F32, tag="ext")
                                out_ap = ext
                            else:
                                out_ap = ps_flat
                            nc.scalar.activation(
                                out=out_ap,
                                in_=ps_flat,
                                func=mybir.ActivationFunctionType.Exp,
                                scale=SCALE,
                                accum_out=(
                                    st[:, idx : idx + 1] if level >= 2 else None
                                ),
                            )
                        if is_dump and level >= 1:
                            q = dump_q
                            lo, hi = DUMPS[q][2], DUMPS[q][3]
                            nc.sync.dma_start(
                                out=extd_d.ap()[:, q * JT + lo : q * JT + hi],
                                in_=ext_b[:, lo:hi],
                            )
                        if win_tile:
                            if split:
                                ext = ps.rearrange("p s j -> p (s j)")
                            wj = junkp.tile([P, w], F32, tag="wj")
                            widx = IC * NJT + ic
                            nc.vector.tensor_scalar(
                                out=wj,
                                in0=ext[:, 0:w],
                                scalar1=1.0,
                                scalar2=0.0,
                                op0=mybir.AluOpType.mult,
                                op1=mybir.AluOpType.add,
                                accum_out=st[:, widx : widx + 1],
                            )
                nc.sync.dma_start(out=partial_d.ap(), in_=st)

    nc.compile()
    if dedupe:
        dedupe_ldweights(nc)
    _build_cache[key] = nc
    return nc


def make_in_maps(embeddings: np.ndarray, labels: np.ndarray, tri=None):
    emb = np.asarray(embeddings, dtype=np.float32)
    lab_f = np.asarray(labels).astype(np.float32)
    perm = np.argsort(-lab_f, kind="stable")
    emb_p = emb[perm]
    n_ref = int(lab_f.sum())
    assert n_ref <= NCORES * RES + 1024, "n_ref exceeds device+host capacity"
    if tri is None:
        tri = n_ref >= NCORES * RES  # triangle math assumes all device rows ref
    embT = np.ascontiguousarray(emb_p.T).astype(F8NP)  # [D, N], one array
    in_maps = []
    for c in range(NCORES):
        if tri:
            # core c owns row-chunks {c, 8+c, 16+c, 24+c} (one per band)
            resT = np.concatenate(
                [
                    embT[:, (8 * ic + c) * P : (8 * ic + c + 1) * P]
                    for ic in range(IC)
                ],
                axis=1,
            )
        else:
            resT = embT[:, c * RES : (c + 1) * RES]
        in_maps.append(
            {
                "resT": np.ascontiguousarray(resT),
                "embT": embT,
            }
        )
    qf32 = embT.astype(np.float32)
    sq = np.einsum("dn,dn->n", qf32, qf32, dtype=np.float32)  # ||q(e_i)||^2
    ctx = {"emb_p": emb_p, "n_ref": n_ref, "sq": sq, "tri": tri}
    return in_maps, ctx


def host_finish(partials, ctx, extds=None):
    """partials: list of [P, NOUT] per core -> scalar loss (f32).
    extds: list of [P, 4*JT] bf16 dump tiles per core (triangle mode)."""
    n_ref = ctx["n_ref"]
    emb_p = ctx["emb_p"]
    sq = ctx["sq"]
    tri = ctx.get("tri", False)
    jtf = n_ref // JT
    dev_rows = NCORES * RES
    nb = IC * NJT + IC
    total = np.float64(0.0)
    missing = None
    if tri:
        # CS[band J, col x] = sum over band-J rows of e_{row, x}, summed
        # across cores; dump quarter q of core c covers (ic, jt) = DUMPS[q]
        CS = np.zeros((IC - 1, dev_rows), np.float64)
        for c in range(NCORES):
            ed = np.asarray(extds[c], np.float32)  # [P, 4*JT]
            for q, (ic, jt, lo, hi) in enumerate(DUMPS):
                cs = ed[:, q * JT + lo : q * JT + hi].sum(axis=0)
                CS[ic, jt * JT + lo : jt * JT + hi] += cs
        # missing(x) = sum_{J < band(x)} CS[J, x]
        band = np.arange(dev_rows) // (8 * P)
        missing = np.zeros(dev_rows, np.float64)
        for J in range(IC - 1):
            missing += np.where(band > J, CS[J], 0.0)
    for c in range(NCORES):
        arr = np.asarray(partials[c], np.float32)
        A = arr[:, : IC * NJT].reshape(P, IC, NJT).copy()
        if tri:
            # first processed tile is (ic0, jt3); no last-tile split
            A[:, 0, NJT - 1] = arr[:, nb : nb + NS].sum(axis=1)
        else:
            A[:, 0, 0] = arr[:, nb : nb + NS].sum(axis=1)        # first tile
            A[:, IC - 1, NJT - 1] = arr[:, nb + NS : nb + 2 * NS].sum(axis=1)
        Wv = arr[:, IC * NJT : nb]                               # [P, IC]
        if tri:
            # fully-skipped tiles' st columns are garbage -> zero them
            for ic in range(IC):
                for jt in range(NJT):
                    if all(_skip(ic, jt, s) for s in range(NS)):
                        A[:, ic, jt] = 0.0
            rows = (8 * np.arange(IC)[None, :] + c) * P + np.arange(P)[:, None]
        else:
            rows = c * RES + np.arange(IC)[None, :] * P + np.arange(P)[:, None]
        corr = np.exp(SCALE * sq[rows])                          # [P, IC]
        all_r = A.sum(axis=2) - corr
        pos_r = A[:, :, :jtf].sum(axis=2) + Wv - corr
        if tri:
            m = missing[rows].astype(np.float32)
            all_r = all_r + m
            pos_r = pos_r + m
        mask = rows < n_ref
        if not mask.any():
            continue
        contrib = np.where(
            mask,
            np.log(np.maximum(all_r, 1e-30) + EPS) - np.log(np.maximum(pos_r, 1e-30)),
            0.0,
        )
        total += contrib.sum(dtype=np.float64)
    if n_ref > dev_rows:
        hr = np.arange(dev_rows, n_ref)
        sim_h = (emb_p[hr] @ emb_p.T) * SCALE
        e_h = np.exp(sim_h)
        diag = e_h[np.arange(len(hr)), hr]
        all_h = e_h.sum(axis=1) - diag
        pos_h = e_h[:, :n_ref].sum(axis=1) - diag
        total += (np.log(all_h + EPS) - np.log(pos_h)).sum(dtype=np.float64)
    loss = total / max(n_ref, 1)
    return np.float32(loss)


def kernel(embeddings: np.ndarray, labels: np.ndarray) -> np.ndarray:
    lab_f = np.asarray(labels).astype(np.float32)
    n_ref = float(lab_f.sum())
    if n_ref < 2:
        return np.float32(0.0)

    in_maps, ctx = make_in_maps(embeddings, labels)
    w = ctx["n_ref"] % JT
    jtf = ctx["n_ref"] // JT
    nc = build(reps=1, w=w, jtf=jtf, tri=ctx["tri"])
    res = run_bass_kernel_spmd(nc, in_maps, core_ids=list(range(NCORES)))
    partials = [res.results[c]["partial"] for c in range(NCORES)]
    extds = (
        [res.results[c]["extd"] for c in range(NCORES)] if ctx["tri"] else None
    )
    return np.asarray(host_finish(partials, ctx, extds), dtype=np.float32)
